# revision 1
# baseline (speedup 1.0000x reference)
"""TRN2 Bass kernel for nn_Attention_5720896438407 (8-core data-parallel).

Mathematical collapse: the module computes SDPA over the *head* axis with a
single KV head (KV=1), so the softmax runs over a size-1 axis and every
attention weight is exactly 1.0.  The q path (q_a/q_norm/q_b), both rotary
embeddings, the nope/rope blend and the attention mask all cancel out, and
the module reduces to

    T  = hidden @ kv_a_w.T + kv_a_b                    # (ntok, 512)
    s  = rsqrt(mean(T^2, -1) + eps)                    # per-token RMS scale
    V  = (s*T) @ (kv_b_w[128:] * (1 + kv_norm_w)).T + kv_b_b[128:]
    Y  = V @ M.T      with  M = o_w.reshape(2048, 16, 128).sum(1)

(the attention output tiles V across all 16 heads, so o_proj sees the head
sum of its weight).  This is what the kernel computes, numerically verified
to ~3e-7 relative error against the full reference in fp64.

Distribution: pure data-parallel over the 8192 tokens — 1024 tokens per
NeuronCore, no collectives.  Per core the tokens stream through in 8 slabs
of 128 tokens; each slab's full pipeline (step-1 matmul, RMS, PE transpose,
V, Y, output DMA) starts as soon as its 0.5 MB input slab lands, so the
input stream (SP HWDGE ring) and output stream (ACT HWDGE ring) overlap for
the whole kernel.  Step-1 operands are fp16 (halves the dominant input
bytes; fp32 PSUM accumulation), the RMS statistics are computed in fp32,
the small downstream matmuls run in fp16, and Y ships as fp16 (host
casts back to fp32), halving the output stream.  End-to-end error vs the
fp32 reference is ~5e-4 relative.
"""
import sys

sys.path.insert(0, "/opt/trn_rl_repo")

import numpy as np
import concourse.bass as bass
import concourse.tile as tile
from concourse import bacc, mybir
from concourse.bass_utils import run_bass_kernel_spmd
from concourse.masks import make_identity

F32 = mybir.dt.float32
F16 = mybir.dt.float16

HID = 2048
KV = 512
D = 128
OUT = 2048
EPS = 1e-6
N_HID_CK = HID // 128   # 16
N_KV_CK = KV // 128     # 4
N_OUT_T = OUT // 512    # 4
SLAB = 128              # tokens per slab
N_CORES = 8
AF = mybir.ActivationFunctionType

_NC_CACHE = {}


def _build_nc(tok, with_ba):
    nslab = tok // SLAB
    assert tok % SLAB == 0

    nc = bacc.Bacc("TRN2", target_bir_lowering=False, debug=False,
                   num_devices=1)

    xts_d = nc.dram_tensor("xts", (nslab, 128, N_HID_CK, SLAB), F16,
                           kind="ExternalInput").ap()
    w1s_d = nc.dram_tensor("w1s", (128, N_HID_CK, KV), F16,
                           kind="ExternalInput").ap()
    wvt_d = nc.dram_tensor("wvt", (KV, D), F16, kind="ExternalInput").ap()
    mt_d = nc.dram_tensor("mt", (D, OUT), F16, kind="ExternalInput").ap()
    bv_d = nc.dram_tensor("bv", (D, 1), F32, kind="ExternalInput").ap()
    if with_ba:
        ba_d = nc.dram_tensor("bar", (1, KV), F16, kind="ExternalInput").ap()
        onesr_d = nc.dram_tensor("onesr", (1, 128), F16,
                                 kind="ExternalInput").ap()
    y_d = nc.dram_tensor("y", (tok, OUT), F16, kind="ExternalOutput").ap()

    with tile.TileContext(nc) as tc:
        with tc.tile_pool(name="consts", bufs=1) as consts, \
             tc.tile_pool(name="slabs", bufs=8) as slabs, \
             tc.tile_pool(name="work", bufs=2) as work, \
             tc.tile_pool(name="ps_t", bufs=3, space="PSUM") as ps_t, \
             tc.tile_pool(name="ps_r", bufs=2, space="PSUM") as ps_r, \
             tc.tile_pool(name="ps_v", bufs=1, space="PSUM") as ps_v, \
             tc.tile_pool(name="ps_y", bufs=2, space="PSUM") as ps_y:
            # ---- input stream on the SP ring, in priority order:
            #      W1 quarters interleaved with slab-0 quarters, then the
            #      remaining slabs (each 0.5 MB, fully contiguous thanks to
            #      the host-side swizzle) ----
            w1_s = consts.tile([128, N_HID_CK, KV], F16, tag="w1")
            sg0 = slabs.tile([128, N_HID_CK, SLAB], F16, tag="slab",
                             name="slab0")
            for h in range(4):
                nc.sync.dma_start(w1_s[:, 4 * h:4 * h + 4, :],
                                  w1s_d[:, 4 * h:4 * h + 4, :])
                nc.sync.dma_start(sg0[:, 4 * h:4 * h + 4, :],
                                  xts_d[0, :, 4 * h:4 * h + 4, :])
            sg = [sg0]
            for g in range(1, nslab):
                t = slabs.tile([128, N_HID_CK, SLAB], F16, tag="slab",
                               name=f"slab{g}")
                nc.sync.dma_start(t[:], xts_d[g])
                sg.append(t)
            # ---- small constants + all output DMAs on the ACT ring ----
            wv_s = []
            for c in range(N_KV_CK):
                t = consts.tile([128, D], F16, tag=f"wv_{c}", name=f"wv_{c}")
                nc.scalar.dma_start(t[:], wvt_d[c * 128:(c + 1) * 128, :])
                wv_s.append(t)
            mt_s = consts.tile([128, OUT], F16, tag="mt")
            nc.scalar.dma_start(mt_s[:], mt_d)
            bv_s = consts.tile([128, 1], F32, tag="bv")
            nc.scalar.dma_start(bv_s[:], bv_d)
            if with_ba:
                ba_s = consts.tile([1, KV], F16, tag="ba")
                nc.scalar.dma_start(ba_s[:], ba_d)
                onesr_s = consts.tile([1, 128], F16, tag="onesr")
                nc.scalar.dma_start(onesr_s[:], onesr_d)
            # ---- PE warm-up: junk matmuls on the (early-ready) identity
            #      keep the HAM activity monitor from throttling the PE
            #      while the first data DMAs are in flight ----
            ident = consts.tile([128, 128], F16, tag="ident")
            make_identity(nc, ident[:])
            js = consts.tile([128, 512], F16, tag="js")
            nc.gpsimd.memset(js[:], 0.0)
            junka = ps_y.tile([128, 512], F32, tag="py", name="junka")
            junkb = ps_y.tile([128, 512], F32, tag="py", name="junkb")
            for i in range(10):
                nc.tensor.matmul(junka[:] if i % 2 == 0 else junkb[:],
                                 ident[:], js[:], start=True, stop=True)
            eps_s = consts.tile([128, 1], F32, tag="eps")
            nc.vector.memset(eps_s[:], EPS)

            def step1(g):
                # T.T slab accumulation, token-major: 16 chunk matmuls,
                # fp16 operands, fp32 PSUM.
                pt = ps_t.tile([128, KV], F32, tag="pt", name=f"pt{g}")
                for ck in range(N_HID_CK):
                    nc.tensor.matmul(
                        pt[:], sg[g][:, ck, :], w1_s[:, ck, :],
                        start=(ck == 0),
                        stop=(ck == N_HID_CK - 1 and not with_ba),
                    )
                if with_ba:
                    # rank-1 row-broadcast of kv_a_b into the accumulation
                    nc.tensor.matmul(pt[:], onesr_s[:], ba_s[:],
                                     start=False, stop=True)
                return pt

            def tail(g, pt):
                t0 = g * SLAB
                # RMS statistics: Square activation with free-axis
                # accumulator gives sum(T^2) per token in one op.
                sqj = work.tile([128, KV], F32, tag="sqj")
                ssq = work.tile([128, 1], F32, tag="ssq")
                nc.scalar.activation(sqj[:], pt[:], AF.Square,
                                     accum_out=ssq[:])
                rt = work.tile([128, 1], F32, tag="rt")
                nc.scalar.activation(rt[:], ssq[:], AF.Sqrt,
                                     bias=eps_s[:], scale=1.0 / KV)
                sc = work.tile([128, 1], F32, tag="sc")
                nc.vector.reciprocal(sc[:], rt[:])
                ttn = work.tile([128, KV], F16, tag="ttn", bufs=3)
                nc.vector.tensor_scalar_mul(ttn[:], pt[:], sc[:])
                # transpose the scaled T into kv-major for step 2
                trp = ps_r.tile([128, N_KV_CK, SLAB], F16, tag="trp",
                                name=f"trp{g}")
                for c in range(N_KV_CK):
                    nc.tensor.transpose(trp[:, c, :],
                                        ttn[:, c * 128:(c + 1) * 128],
                                        ident[:])
                ttr = work.tile([128, N_KV_CK, SLAB], F16, tag="ttr", bufs=3)
                nc.vector.tensor_copy(ttr[:], trp[:])
                # step 2: V.T = Wv' @ (sT).T, bias kv_b_b on the copy
                vtp = ps_v.tile([128, SLAB], F32, tag="vtp", name=f"vtp{g}")
                for c in range(N_KV_CK):
                    nc.tensor.matmul(vtp[:], wv_s[c][:], ttr[:, c, :],
                                     start=(c == 0),
                                     stop=(c == N_KV_CK - 1))
                vts = work.tile([128, SLAB], F16, tag="vts", bufs=3)
                nc.scalar.activation(vts[:], vtp[:], AF.Identity,
                                     bias=bv_s[:], scale=1.0)
                # step 4: Y = V @ M.T, plain PSUM->SBUF copies, 1 MB DMA out
                ysb = work.tile([128, OUT], F16, tag="ysb", bufs=6)
                for n in range(N_OUT_T):
                    py = ps_y.tile([128, 512], F32, tag="py",
                                   name=f"py{g}_{n}")
                    nc.tensor.matmul(py[:], vts[:],
                                     mt_s[:, n * 512:(n + 1) * 512],
                                     start=True, stop=True)
                    ysl = ysb[:, n * 512:(n + 1) * 512]
                    if n % 2 == 0:
                        nc.vector.tensor_copy(ysl, py[:])
                    else:
                        nc.scalar.activation(ysl, py[:], AF.Copy,
                                             bias=0.0, scale=1.0)
                    if g == nslab - 1 and n == 1:
                        # final slab: overlap the first output half with the
                        # remaining matmuls/copies so only 0.25 MB trails
                        nc.scalar.dma_start(y_d[t0:t0 + SLAB, 0:1024],
                                            ysb[:, 0:1024])
                if g == nslab - 1:
                    nc.scalar.dma_start(y_d[t0:t0 + SLAB, 1024:2048],
                                        ysb[:, 1024:2048])
                else:
                    nc.scalar.dma_start(y_d[t0:t0 + SLAB, :], ysb[:])

            # 2-stage software pipeline: slab g's tail is emitted after slab
            # g+1's step-1 matmuls so the PE never waits on the RMS chain.
            prev = None
            for g in range(nslab):
                pt = step1(g)
                if prev is not None:
                    tail(*prev)
                prev = (g, pt)
            tail(*prev)

    nc.compile()
    return nc


def _host_prep(inputs):
    """Fold weights, swizzle X into fp16 token slabs, shard across cores."""
    h = np.asarray(inputs["hidden_states"], dtype=np.float32)
    b, s, hid = h.shape
    assert hid == HID
    x = np.ascontiguousarray(h.reshape(b * s, hid))
    ntok = b * s
    tok = ntok // N_CORES
    nslab = tok // SLAB

    kv_a_w = np.asarray(inputs["kv_a_w"], np.float32)
    kv_a_b = np.asarray(inputs["kv_a_b"], np.float32)
    kv_norm_w = np.asarray(inputs["kv_norm_w"], np.float32)
    kv_b_w = np.asarray(inputs["kv_b_w"], np.float32)
    kv_b_b = np.asarray(inputs["kv_b_b"], np.float32)
    o_w = np.asarray(inputs["o_w"], np.float32)

    w1s = np.ascontiguousarray(
        kv_a_w.T.reshape(N_HID_CK, 128, KV).transpose(1, 0, 2)
    ).astype(np.float16)
    wv = kv_b_w[D:2 * D] * (1.0 + kv_norm_w)[None, :]
    wvt = np.ascontiguousarray(wv.T).astype(np.float16)
    M = o_w.reshape(HID, 16, D).sum(axis=1)
    mt = np.ascontiguousarray(M.T).astype(np.float16)
    bv = np.ascontiguousarray(kv_b_b[D:2 * D].reshape(D, 1)).astype(np.float32)
    with_ba = bool(np.any(kv_a_b != 0.0))
    ba_row = np.ascontiguousarray(kv_a_b.reshape(1, KV)).astype(np.float16)
    ones_row = np.ones((1, 128), np.float16)

    in_maps = []
    for i in range(N_CORES):
        shard = x[i * tok:(i + 1) * tok]
        xts = np.ascontiguousarray(
            shard.T.reshape(N_HID_CK, 128, nslab, SLAB).transpose(2, 1, 0, 3)
        ).astype(np.float16)
        m = {"xts": xts, "w1s": w1s, "wvt": wvt, "mt": mt, "bv": bv}
        if with_ba:
            m["bar"] = ba_row
            m["onesr"] = ones_row
        in_maps.append(m)

    def gather(results):
        y = np.concatenate([r["y"] for r in results], axis=0)
        return np.ascontiguousarray(y.reshape(b, s, HID).astype(np.float32))

    return in_maps, gather, with_ba, tok


def _run(inputs, trace=False, **spmd_kwargs):
    in_maps, gather, with_ba, tok = _host_prep(inputs)
    key = (tok, with_ba)
    if key not in _NC_CACHE:
        _NC_CACHE[key] = _build_nc(tok, with_ba)
    nc = _NC_CACHE[key]
    res = run_bass_kernel_spmd(nc, in_maps, core_ids=list(range(N_CORES)),
                               trace=trace, **spmd_kwargs)
    return gather(res.results), res


def kernel(**inputs) -> np.ndarray:
    y, _ = _run(inputs, trace=False)
    return y



# revision 2
# speedup vs baseline: 1.1494x; 1.1494x over previous
"""TRN2 Bass kernel for nn_Attention_5720896438407 (8-core data-parallel), v3.

Math (see the derivation in the original kernel): the module collapses to

    T  = hidden @ kv_a_w.T (+ kv_a_b)            # (ntok, 512), stats only
    s  = rsqrt(mean(T^2, -1) + eps)              # per-token RMS scale
    V' = hidden @ Wc.T (+ cv)                    # Wc = Wv' @ W1  (128, 2048)
    Y  = (s * V') @ M.T (+ yb)                   # M = o_w.reshape(2048,16,128).sum(1)

where Wv' = kv_b_w[128:256] * (1 + kv_norm_w).  Folding Wc := Wv' @ W1 lets
the V path go straight from the input (no transposes, no dependency on the
RMS statistics); the per-token scale s is applied on the PSUM->SBUF copy of
the final matmul output (token-major there).  T is needed ONLY for the RMS
statistics, whose 512-wide mean-square averages away elementwise quantization
noise, so that matmul runs in fp8 (DoubleRow, 2x PE throughput) with ~0.2%
impact on the final output (measured rel_fro ~2e-3 vs the 2e-2 gate).

Engine plan per 128-token slab (steady state):
  PE   : 16 MM N=128 (V path, fp16) + 8 DR MM N=512 (stats, fp8)
         + 4 MM N=512 (output)                      ~3.7us  <- critical
  DVE  : fp16->fp8 cast of the next X slab + reciprocal + V.T copy
  ACT  : Square+accum, Sqrt, both scaled PSUM->SBUF output copies (FD=1024)
  SP   : all input DMA issues, then all output DMA issues (~0.8us each)
  GpSimd: idle (its fp8 cast measured 29 Gelem/s -- far too slow)
step4 for slab g is emitted after step1 of slab g+1 so its scaled copies
never wait on the stats chain.
"""
import sys

sys.path.insert(0, "/opt/trn_rl_repo")

import numpy as np
import ml_dtypes
import concourse.bass as bass
import concourse.tile as tile
from concourse import bacc, mybir
from concourse.bass_utils import run_bass_kernel_spmd

F32 = mybir.dt.float32
F16 = mybir.dt.float16
F8 = mybir.dt.float8e4
NP_F8 = ml_dtypes.float8_e4m3

HID = 2048
KV = 512
D = 128
OUT = 2048
EPS = 1e-6
NCK = HID // 128         # 16 hid chunks
SLAB = 128               # tokens per slab
N_CORES = 8
WSCALE = 256.0           # fp8 weight pre-scale for the stats matmul
AF = mybir.ActivationFunctionType
DR = mybir.MatmulPerfMode.DoubleRow

_NC_CACHE = {}


def _build_nc(tok, with_ba, with_bv, use_fp8=True):
    nslab = tok // SLAB
    assert tok % SLAB == 0

    nc = bacc.Bacc("TRN2", target_bir_lowering=False, debug=False,
                   num_devices=1)

    xts_d = nc.dram_tensor("xts", (nslab, 128, NCK, SLAB), F16,
                           kind="ExternalInput").ap()
    w1s_d = nc.dram_tensor("w1s", (128, NCK, KV), F8 if use_fp8 else F16,
                           kind="ExternalInput").ap()
    wct_d = nc.dram_tensor("wct", (128, NCK, D), F16,
                           kind="ExternalInput").ap()
    mt_d = nc.dram_tensor("mt", (D, OUT), F16, kind="ExternalInput").ap()
    if with_ba:
        ba_d = nc.dram_tensor("bar", (1, KV), F16, kind="ExternalInput").ap()
        onesr_d = nc.dram_tensor("onesr", (1, 128), F16,
                                 kind="ExternalInput").ap()
        cv_d = nc.dram_tensor("cv", (D, 1), F32, kind="ExternalInput").ap()
    if with_bv:
        ybb_d = nc.dram_tensor("ybb", (128, OUT), F16,
                               kind="ExternalInput").ap()
    y_d = nc.dram_tensor("y", (tok, OUT), F16, kind="ExternalOutput").ap()

    with tile.TileContext(nc) as tc:
        with tc.tile_pool(name="consts", bufs=1) as consts, \
             tc.tile_pool(name="slabs", bufs=8) as slabs, \
             tc.tile_pool(name="x8p", bufs=3) as x8p, \
             tc.tile_pool(name="work", bufs=2) as work, \
             tc.tile_pool(name="ps_t", bufs=2, space="PSUM") as ps_t, \
             tc.tile_pool(name="ps_v", bufs=2, space="PSUM") as ps_v, \
             tc.tile_pool(name="ps_y", bufs=2, space="PSUM") as ps_y:

            # ---- input stream on the SP ring, in the order the PE can
            #      consume it: x0 (V path first), then W1 halves woven
            #      between the next slabs.  All output DMAs are issued
            #      from SP too (each dma_start costs ~0.8us of
            #      issuing-engine time; SP is otherwise idle). ----
            w1_s = consts.tile([128, NCK, KV], F8 if use_fp8 else F16,
                               tag="w1")
            sg = [slabs.tile([128, NCK, SLAB], F16, tag="slab",
                             name=f"slab{g}") for g in range(nslab)]
            nc.sync.dma_start(sg[0][:], xts_d[0])
            nc.sync.dma_start(w1_s[:, 0:8, :], w1s_d[:, 0:8, :])
            nc.sync.dma_start(sg[1][:], xts_d[1])
            nc.sync.dma_start(w1_s[:, 8:16, :], w1s_d[:, 8:16, :])
            for g in range(2, nslab):
                nc.sync.dma_start(sg[g][:], xts_d[g])

            # ---- weight constants on the ACT ring ----
            wc_s = consts.tile([128, NCK, D], F16, tag="wc")
            nc.gpsimd.dma_start(wc_s[:], wct_d)
            mt_s = consts.tile([128, OUT], F16, tag="mt")
            nc.gpsimd.dma_start(mt_s[:], mt_d)
            if with_ba:
                ba_s = consts.tile([1, KV], F16, tag="ba")
                nc.gpsimd.dma_start(ba_s[:], ba_d)
                onesr_s = consts.tile([1, 128], F16, tag="onesr")
                nc.gpsimd.dma_start(onesr_s[:], onesr_d)
                cv_s = consts.tile([128, 1], F32, tag="cv")
                nc.gpsimd.dma_start(cv_s[:], cv_d)
            if with_bv:
                ybb_s = consts.tile([128, OUT], F16, tag="ybb")
                nc.gpsimd.dma_start(ybb_s[:], ybb_d)

            # ---- PE warm-up: junk matmuls on a zero tile (no DMA deps) ----
            js = consts.tile([128, 512], F16, tag="js")
            nc.vector.memset(js[:], 0.0)
            junka = ps_y.tile([128, 1024], F32, tag="py", name="junka")
            junkb = ps_y.tile([128, 1024], F32, tag="py", name="junkb")
            for i in range(5):
                nc.tensor.matmul((junka if i % 2 == 0 else junkb)[:, 0:512],
                                 js[:, 0:128], js[:], start=True, stop=True)
            eps_s = consts.tile([128, 1], F32, tag="eps")
            nc.vector.memset(eps_s[:], EPS)

            x8 = []
            if use_fp8:
                for g in range(nslab):
                    x8.append(x8p.tile([128, NCK, SLAB], F8, tag="x8",
                                       name=f"x8_{g}", bufs=3))

            def cast_slab(g, half=None):
                # fp16 -> fp8 on DVE (vector); gpsimd measured 4x slower
                if not use_fp8:
                    return
                if half is None:
                    nc.vector.tensor_copy(x8[g][:], sg[g][:])
                else:
                    h = half
                    nc.vector.tensor_copy(x8[g][:, 8 * h:8 * h + 8, :],
                                          sg[g][:, 8 * h:8 * h + 8, :])

            def step1_mm(pt, g, k):
                nc.tensor.matmul(
                    pt[:], x8[g][:, 2 * k:2 * k + 2, :],
                    w1_s[:, 2 * k:2 * k + 2, :],
                    start=(k == 0),
                    stop=(k == NCK // 2 - 1 and not with_ba),
                    perf_mode=DR,
                )

            def step1(g):
                # stats path: pt ~= WSCALE * T, fp8 DoubleRow
                pt = ps_t.tile([128, KV], F32, tag="pt", name=f"pt{g}")
                if use_fp8:
                    for k in range(NCK // 2):
                        step1_mm(pt, g, k)
                else:
                    for ck in range(NCK):
                        nc.tensor.matmul(
                            pt[:], sg[g][:, ck, :], w1_s[:, ck, :],
                            start=(ck == 0),
                            stop=(ck == NCK - 1 and not with_ba),
                        )
                if with_ba:
                    nc.tensor.matmul(pt[:], onesr_s[:], ba_s[:],
                                     start=False, stop=True)
                return pt

            def step2p(g):
                # V path: vt = Wc @ X.T (vd-major), fp16, indep of stats
                vt = ps_v.tile([128, 512], F32, tag="vt", name=f"vt{g}")
                for ck in range(NCK):
                    nc.tensor.matmul(vt[:, 0:SLAB], wc_s[:, ck, :],
                                     sg[g][:, ck, :],
                                     start=(ck == 0), stop=(ck == NCK - 1))
                return vt

            def stats(g, pt):
                dq = (1.0 / WSCALE) if use_fp8 else 1.0
                sqj = work.tile([128, KV], F16, tag="sqj")
                ssq = work.tile([128, 1], F32, tag="ssq")
                nc.scalar.activation(sqj[:], pt[:], AF.Square,
                                     scale=dq, accum_out=ssq[:])
                rt = work.tile([128, 1], F32, tag="rt")
                nc.scalar.activation(rt[:], ssq[:], AF.Sqrt,
                                     bias=eps_s[:], scale=1.0 / KV)
                sc = work.tile([128, 1], F32, tag="sc", bufs=3)
                nc.vector.reciprocal(sc[:], rt[:])
                return sc

            def vcopy(g, vt):
                vts = work.tile([128, SLAB], F16, tag="vts", bufs=3)
                if with_ba:
                    nc.vector.tensor_scalar_add(vts[:], vt[:, 0:SLAB],
                                                cv_s[:])
                else:
                    nc.vector.tensor_copy(vts[:], vt[:, 0:SLAB])
                return vts

            def step4(g, vts, sc, tail):
                t0 = g * SLAB
                ysb = work.tile([128, OUT], F16, tag="ysb", bufs=4)
                for p in range(2):         # two FD=1024 halves
                    py = ps_y.tile([128, 1024], F32, tag="py",
                                   name=f"py{g}_{p}")
                    if tail:
                        # quarter-pipelined tail: MM -> scaled copy -> DMA
                        # per 512 columns, DVE/ACT alternating, so the
                        # final DMA departs as early as possible
                        for n in range(2):
                            c0 = p * 1024 + n * 512
                            pyq = py[:, n * 512:(n + 1) * 512]
                            nc.tensor.matmul(pyq, vts[:],
                                             mt_s[:, c0:c0 + 512],
                                             start=True, stop=True)
                            ysq = ysb[:, c0:c0 + 512]
                            if n == 0:
                                nc.vector.tensor_scalar_mul(ysq, pyq, sc[:])
                            else:
                                nc.scalar.activation(ysq, pyq, AF.Copy,
                                                     bias=0.0, scale=sc[:])
                            if with_bv:
                                nc.vector.tensor_add(
                                    ysq, ysq, ybb_s[:, c0:c0 + 512])
                            nc.sync.dma_start(y_d[t0:t0 + SLAB, c0:c0 + 512],
                                              ysq)
                        continue
                    for n in range(2):
                        nc.tensor.matmul(
                            py[:, n * 512:(n + 1) * 512], vts[:],
                            mt_s[:, (2 * p + n) * 512:(2 * p + n + 1) * 512],
                            start=True, stop=True)
                    # scaled PSUM->SBUF copy: half on DVE, half on ACT so
                    # neither engine serializes the slab
                    ysl = ysb[:, p * 1024:(p + 1) * 1024]
                    if p == 0:
                        nc.vector.tensor_scalar_mul(ysl, py[:], sc[:])
                    else:
                        nc.scalar.activation(ysl, py[:], AF.Copy,
                                             bias=0.0, scale=sc[:])
                    if with_bv:
                        nc.vector.tensor_add(
                            ysl, ysl, ybb_s[:, p * 1024:(p + 1) * 1024])
                if not tail:
                    nc.sync.dma_start(y_d[t0:t0 + SLAB, :], ysb[:])

            # ---- prologue: data-greedy order matching the input stream
            #      (x0, w1h0, x1, w1h1): V path of slab 0 first (needs
            #      only x0+wc), stats halves woven between, slab 1's V
            #      path filling the w1h1 wait ----
            vt_pre = {}
            if use_fp8:
                cast_slab(0)
                vt0 = step2p(0)
                pt0 = ps_t.tile([128, KV], F32, tag="pt", name="pt0")
                for k in range(0, 4):          # needs w1 half 0
                    step1_mm(pt0, 0, k)
                cast_slab(1)
                vt_pre[1] = step2p(1)
                for k in range(4, 8):          # needs w1 half 1
                    step1_mm(pt0, 0, k)
                if with_ba:
                    nc.tensor.matmul(pt0[:], onesr_s[:], ba_s[:],
                                     start=False, stop=True)
            else:
                pt0 = step1(0)
                vt0 = step2p(0)
                cast_slab(1)
            vts = vcopy(0, vt0)
            sc = stats(0, pt0)
            prev = 0

            # ---- steady state: PE order [step2p(g), step4(g-1),
            #      step1(g)] -- step4 only needs vts/mt (never DMA-gated)
            #      and its scaled copies catch sc(g-1) just in time.
            #      vcopy is emitted before stats so it isn't stuck behind
            #      the reciprocal on the DVE FIFO. ----
            for g in range(1, nslab):
                if g == nslab - 1:
                    # last slab: stats first so the tail waits less
                    pt = step1(g)
                    sc_n = stats(g, pt)
                    vt = step2p(g)
                    step4(prev, vts, sc, tail=False)
                    sc = sc_n
                    vts = vcopy(g, vt)
                else:
                    vt = vt_pre.pop(g, None)
                    if vt is None:
                        vt = step2p(g)
                    step4(prev, vts, sc, tail=False)
                    pt = step1(g)
                    cast_slab(g + 1)
                    vts = vcopy(g, vt)
                    sc = stats(g, pt)
                prev = g
            step4(prev, vts, sc, tail=True)

    nc.compile()
    return nc


def _host_prep(inputs, use_fp8=True):
    """Fold weights, swizzle X into fp16 token slabs, shard across cores."""
    h = np.asarray(inputs["hidden_states"], dtype=np.float32)
    b, s, hid = h.shape
    assert hid == HID
    x = np.ascontiguousarray(h.reshape(b * s, hid))
    ntok = b * s
    tok = ntok // N_CORES
    nslab = tok // SLAB

    kv_a_w = np.asarray(inputs["kv_a_w"], np.float32)
    kv_a_b = np.asarray(inputs["kv_a_b"], np.float32)
    kv_norm_w = np.asarray(inputs["kv_norm_w"], np.float32)
    kv_b_w = np.asarray(inputs["kv_b_w"], np.float32)
    kv_b_b = np.asarray(inputs["kv_b_b"], np.float32)
    o_w = np.asarray(inputs["o_w"], np.float32)

    wv = kv_b_w[D:2 * D] * (1.0 + kv_norm_w)[None, :]       # (128, 512)
    wc = wv @ kv_a_w                                        # (128, 2048)
    M = o_w.reshape(HID, 16, D).sum(axis=1)                 # (2048, 128)

    w1t = kv_a_w.T.reshape(NCK, 128, KV).transpose(1, 0, 2)
    if use_fp8:
        w1s = np.ascontiguousarray(w1t * WSCALE).astype(NP_F8)
    else:
        w1s = np.ascontiguousarray(w1t).astype(np.float16)
    wct = np.ascontiguousarray(
        wc.T.reshape(NCK, 128, D).transpose(1, 0, 2)
    ).astype(np.float16)
    mt = np.ascontiguousarray(M.T).astype(np.float16)

    with_ba = bool(np.any(kv_a_b != 0.0))
    with_bv = bool(np.any(kv_b_b[D:2 * D] != 0.0))
    ba_row = np.ascontiguousarray(
        (kv_a_b * (WSCALE if use_fp8 else 1.0)).reshape(1, KV)
    ).astype(np.float16)
    ones_row = np.ones((1, 128), np.float16)
    cv = np.ascontiguousarray((wv @ kv_a_b).reshape(D, 1)).astype(np.float32)
    yb = M @ kv_b_b[D:2 * D]                                # (2048,)
    ybb = np.ascontiguousarray(
        np.broadcast_to(yb[None, :], (128, OUT))).astype(np.float16)

    in_maps = []
    for i in range(N_CORES):
        shard = x[i * tok:(i + 1) * tok]
        xts = np.ascontiguousarray(
            shard.T.reshape(NCK, 128, nslab, SLAB).transpose(2, 1, 0, 3)
        ).astype(np.float16)
        m = {"xts": xts, "w1s": w1s, "wct": wct, "mt": mt}
        if with_ba:
            m["bar"] = ba_row
            m["onesr"] = ones_row
            m["cv"] = cv
        if with_bv:
            m["ybb"] = ybb
        in_maps.append(m)

    def gather(results):
        y = np.concatenate([r["y"] for r in results], axis=0)
        return np.ascontiguousarray(y.reshape(b, s, HID).astype(np.float32))

    return in_maps, gather, with_ba, with_bv, tok


def _run(inputs, trace=False, use_fp8=True, **spmd_kwargs):
    in_maps, gather, with_ba, with_bv, tok = _host_prep(inputs, use_fp8)
    key = (tok, with_ba, with_bv, use_fp8)
    if key not in _NC_CACHE:
        _NC_CACHE[key] = _build_nc(tok, with_ba, with_bv, use_fp8)
    nc = _NC_CACHE[key]
    res = run_bass_kernel_spmd(nc, in_maps, core_ids=list(range(N_CORES)),
                               trace=trace, **spmd_kwargs)
    return gather(res.results), res


def kernel(**inputs) -> np.ndarray:
    y, _ = _run(inputs, trace=False)
    return y


# revision 3
# speedup vs baseline: 1.1513x; 1.0017x over previous
"""TRN2 Bass kernel for nn_Attention_5720896438407 (8-core data-parallel), v3.

Math (see the derivation in the original kernel): the module collapses to

    T  = hidden @ kv_a_w.T (+ kv_a_b)            # (ntok, 512), stats only
    s  = rsqrt(mean(T^2, -1) + eps)              # per-token RMS scale
    V' = hidden @ Wc.T (+ cv)                    # Wc = Wv' @ W1  (128, 2048)
    Y  = (s * V') @ M.T (+ yb)                   # M = o_w.reshape(2048,16,128).sum(1)

where Wv' = kv_b_w[128:256] * (1 + kv_norm_w).  Folding Wc := Wv' @ W1 lets
the V path go straight from the input (no transposes, no dependency on the
RMS statistics); the per-token scale s is applied on the PSUM->SBUF copy of
the final matmul output (token-major there).  T is needed ONLY for the RMS
statistics, whose 512-wide mean-square averages away elementwise quantization
noise, so that matmul runs in fp8 (DoubleRow, 2x PE throughput) with ~0.2%
impact on the final output (measured rel_fro ~2e-3 vs the 2e-2 gate).

Engine plan per 128-token slab (steady state):
  PE   : 16 MM N=128 (V path, fp16) + 8 DR MM N=512 (stats, fp8)
         + 4 MM N=512 (output)                      ~3.7us  <- critical
  DVE  : fp16->fp8 cast of the next X slab + reciprocal + V.T copy
  ACT  : Square+accum, Sqrt, both scaled PSUM->SBUF output copies (FD=1024)
  SP   : all input DMA issues, then all output DMA issues (~0.8us each)
  GpSimd: idle (its fp8 cast measured 29 Gelem/s -- far too slow)
step4 for slab g is emitted after step1 of slab g+1 so its scaled copies
never wait on the stats chain.
"""
import sys

sys.path.insert(0, "/opt/trn_rl_repo")

import numpy as np
import ml_dtypes
import concourse.bass as bass
import concourse.tile as tile
from concourse import bacc, mybir
from concourse.bass_utils import run_bass_kernel_spmd

F32 = mybir.dt.float32
F16 = mybir.dt.float16
F8 = mybir.dt.float8e4
NP_F8 = ml_dtypes.float8_e4m3

HID = 2048
KV = 512
D = 128
OUT = 2048
EPS = 1e-6
NCK = HID // 128         # 16 hid chunks
SLAB = 128               # tokens per slab
N_CORES = 8
WSCALE = 256.0           # fp8 weight pre-scale for the stats matmul
AF = mybir.ActivationFunctionType
DR = mybir.MatmulPerfMode.DoubleRow

_NC_CACHE = {}


def _build_nc(tok, with_ba, with_bv, use_fp8=True):
    nslab = tok // SLAB
    assert tok % SLAB == 0

    nc = bacc.Bacc("TRN2", target_bir_lowering=False, debug=False,
                   num_devices=1)

    xts_d = nc.dram_tensor("xts", (nslab, 128, NCK, SLAB), F16,
                           kind="ExternalInput").ap()
    w1s_d = nc.dram_tensor("w1s", (128, NCK, KV), F8 if use_fp8 else F16,
                           kind="ExternalInput").ap()
    wct_d = nc.dram_tensor("wct", (128, NCK, D), F16,
                           kind="ExternalInput").ap()
    mt_d = nc.dram_tensor("mt", (D, OUT), F16, kind="ExternalInput").ap()
    if with_ba:
        ba_d = nc.dram_tensor("bar", (1, KV), F16, kind="ExternalInput").ap()
        onesr_d = nc.dram_tensor("onesr", (1, 128), F16,
                                 kind="ExternalInput").ap()
        cv_d = nc.dram_tensor("cv", (D, 1), F32, kind="ExternalInput").ap()
    if with_bv:
        ybb_d = nc.dram_tensor("ybb", (128, OUT), F16,
                               kind="ExternalInput").ap()
    y_d = nc.dram_tensor("y", (tok, OUT), F16, kind="ExternalOutput").ap()

    with tile.TileContext(nc) as tc:
        with tc.tile_pool(name="consts", bufs=1) as consts, \
             tc.tile_pool(name="slabs", bufs=8) as slabs, \
             tc.tile_pool(name="x8p", bufs=3) as x8p, \
             tc.tile_pool(name="work", bufs=2) as work, \
             tc.tile_pool(name="ps_t", bufs=2, space="PSUM") as ps_t, \
             tc.tile_pool(name="ps_v", bufs=2, space="PSUM") as ps_v, \
             tc.tile_pool(name="ps_y", bufs=2, space="PSUM") as ps_y:

            # ---- input stream on the SP ring, in the order the PE can
            #      consume it: x0 (V path first), then W1 halves woven
            #      between the next slabs.  All output DMAs are issued
            #      from SP too (each dma_start costs ~0.8us of
            #      issuing-engine time; SP is otherwise idle). ----
            w1_s = consts.tile([128, NCK, KV], F8 if use_fp8 else F16,
                               tag="w1")
            sg = [slabs.tile([128, NCK, SLAB], F16, tag="slab",
                             name=f"slab{g}") for g in range(nslab)]
            nc.sync.dma_start(sg[0][:], xts_d[0])
            nc.sync.dma_start(w1_s[:, 0:8, :], w1s_d[:, 0:8, :])
            nc.sync.dma_start(sg[1][:], xts_d[1])
            nc.sync.dma_start(w1_s[:, 8:16, :], w1s_d[:, 8:16, :])
            for g in range(2, nslab):
                nc.sync.dma_start(sg[g][:], xts_d[g])

            # ---- weight constants on the ACT ring ----
            wc_s = consts.tile([128, NCK, D], F16, tag="wc")
            nc.gpsimd.dma_start(wc_s[:], wct_d)
            mt_s = consts.tile([128, OUT], F16, tag="mt")
            nc.gpsimd.dma_start(mt_s[:], mt_d)
            if with_ba:
                ba_s = consts.tile([1, KV], F16, tag="ba")
                nc.gpsimd.dma_start(ba_s[:], ba_d)
                onesr_s = consts.tile([1, 128], F16, tag="onesr")
                nc.gpsimd.dma_start(onesr_s[:], onesr_d)
                cv_s = consts.tile([128, 1], F32, tag="cv")
                nc.gpsimd.dma_start(cv_s[:], cv_d)
            if with_bv:
                ybb_s = consts.tile([128, OUT], F16, tag="ybb")
                nc.gpsimd.dma_start(ybb_s[:], ybb_d)

            # ---- PE warm-up: junk matmuls on a zero tile (no DMA deps) ----
            js = consts.tile([128, 512], F16, tag="js")
            nc.vector.memset(js[:], 0.0)
            junka = ps_y.tile([128, 1024], F32, tag="py", name="junka")
            junkb = ps_y.tile([128, 1024], F32, tag="py", name="junkb")
            for i in range(5):
                nc.tensor.matmul((junka if i % 2 == 0 else junkb)[:, 0:512],
                                 js[:, 0:128], js[:], start=True, stop=True)
            eps_s = consts.tile([128, 1], F32, tag="eps")
            nc.vector.memset(eps_s[:], EPS)

            x8 = []
            if use_fp8:
                for g in range(nslab):
                    x8.append(x8p.tile([128, NCK, SLAB], F8, tag="x8",
                                       name=f"x8_{g}", bufs=3))

            def cast_slab(g, half=None):
                # fp16 -> fp8 on DVE (vector); gpsimd measured 4x slower
                if not use_fp8:
                    return
                if half is None:
                    nc.vector.tensor_copy(x8[g][:], sg[g][:])
                else:
                    h = half
                    nc.vector.tensor_copy(x8[g][:, 8 * h:8 * h + 8, :],
                                          sg[g][:, 8 * h:8 * h + 8, :])

            def step1_mm(pt, g, k):
                nc.tensor.matmul(
                    pt[:], x8[g][:, 2 * k:2 * k + 2, :],
                    w1_s[:, 2 * k:2 * k + 2, :],
                    start=(k == 0),
                    stop=(k == NCK // 2 - 1 and not with_ba),
                    perf_mode=DR,
                )

            def step1(g):
                # stats path: pt ~= WSCALE * T, fp8 DoubleRow
                pt = ps_t.tile([128, KV], F32, tag="pt", name=f"pt{g}")
                if use_fp8:
                    for k in range(NCK // 2):
                        step1_mm(pt, g, k)
                else:
                    for ck in range(NCK):
                        nc.tensor.matmul(
                            pt[:], sg[g][:, ck, :], w1_s[:, ck, :],
                            start=(ck == 0),
                            stop=(ck == NCK - 1 and not with_ba),
                        )
                if with_ba:
                    nc.tensor.matmul(pt[:], onesr_s[:], ba_s[:],
                                     start=False, stop=True)
                return pt

            def step2p(g):
                # V path: vt = Wc @ X.T (vd-major), fp16, indep of stats
                vt = ps_v.tile([128, 512], F32, tag="vt", name=f"vt{g}")
                for ck in range(NCK):
                    nc.tensor.matmul(vt[:, 0:SLAB], wc_s[:, ck, :],
                                     sg[g][:, ck, :],
                                     start=(ck == 0), stop=(ck == NCK - 1))
                return vt

            def stats(g, pt):
                dq = (1.0 / WSCALE) if use_fp8 else 1.0
                sqj = work.tile([128, KV], F16, tag="sqj")
                ssq = work.tile([128, 1], F32, tag="ssq")
                nc.scalar.activation(sqj[:], pt[:], AF.Square,
                                     scale=dq, accum_out=ssq[:])
                rt = work.tile([128, 1], F32, tag="rt")
                nc.scalar.activation(rt[:], ssq[:], AF.Sqrt,
                                     bias=eps_s[:], scale=1.0 / KV)
                sc = work.tile([128, 1], F32, tag="sc", bufs=3)
                nc.vector.reciprocal(sc[:], rt[:])
                return sc

            def vcopy(g, vt):
                vts = work.tile([128, SLAB], F16, tag="vts", bufs=3)
                if with_ba:
                    nc.vector.tensor_scalar_add(vts[:], vt[:, 0:SLAB],
                                                cv_s[:])
                else:
                    nc.vector.tensor_copy(vts[:], vt[:, 0:SLAB])
                return vts

            def step4(g, vts, sc, tail):
                t0 = g * SLAB
                ysb = work.tile([128, OUT], F16, tag="ysb", bufs=4)
                if tail:
                    # quarter-pipelined tail: MM -> scaled copy -> DMA per
                    # 512 columns, DVE/ACT alternating.  The first two
                    # quarters borrow the (already-drained) stats PSUM
                    # banks so nothing waits on the previous slab's
                    # output copies to free ps_y.
                    for q in range(4):
                        c0 = q * 512
                        if q < 2:
                            pyq = ps_t.tile([128, 512], F32, tag="pt",
                                            name=f"pyt{g}_{q}")[:]
                        else:
                            if q == 2:
                                pyt = ps_y.tile([128, 1024], F32, tag="py",
                                                name=f"py{g}_t")
                            pyq = pyt[:, (q - 2) * 512:(q - 1) * 512]
                        nc.tensor.matmul(pyq, vts[:], mt_s[:, c0:c0 + 512],
                                         start=True, stop=True)
                        ysq = ysb[:, c0:c0 + 512]
                        if q % 2 == 0:
                            nc.vector.tensor_scalar_mul(ysq, pyq, sc[:])
                        else:
                            nc.scalar.activation(ysq, pyq, AF.Copy,
                                                 bias=0.0, scale=sc[:])
                        if with_bv:
                            nc.vector.tensor_add(
                                ysq, ysq, ybb_s[:, c0:c0 + 512])
                        nc.sync.dma_start(y_d[t0:t0 + SLAB, c0:c0 + 512],
                                          ysq)
                    return
                for p in range(2):         # two FD=1024 halves
                    py = ps_y.tile([128, 1024], F32, tag="py",
                                   name=f"py{g}_{p}")
                    for n in range(2):
                        nc.tensor.matmul(
                            py[:, n * 512:(n + 1) * 512], vts[:],
                            mt_s[:, (2 * p + n) * 512:(2 * p + n + 1) * 512],
                            start=True, stop=True)
                    # scaled PSUM->SBUF copy: half on DVE, half on ACT so
                    # neither engine serializes the slab
                    ysl = ysb[:, p * 1024:(p + 1) * 1024]
                    if p == 0:
                        nc.vector.tensor_scalar_mul(ysl, py[:], sc[:])
                    else:
                        nc.scalar.activation(ysl, py[:], AF.Copy,
                                             bias=0.0, scale=sc[:])
                    if with_bv:
                        nc.vector.tensor_add(
                            ysl, ysl, ybb_s[:, p * 1024:(p + 1) * 1024])
                if not tail:
                    nc.sync.dma_start(y_d[t0:t0 + SLAB, :], ysb[:])

            # ---- prologue: data-greedy order matching the input stream
            #      (x0, w1h0, x1, w1h1): V path of slab 0 first (needs
            #      only x0+wc), stats halves woven between, slab 1's V
            #      path filling the w1h1 wait ----
            vt_pre = {}
            if use_fp8:
                cast_slab(0)
                vt0 = step2p(0)
                pt0 = ps_t.tile([128, KV], F32, tag="pt", name="pt0")
                for k in range(0, 4):          # needs w1 half 0
                    step1_mm(pt0, 0, k)
                cast_slab(1)
                vt_pre[1] = step2p(1)
                for k in range(4, 8):          # needs w1 half 1
                    step1_mm(pt0, 0, k)
                if with_ba:
                    nc.tensor.matmul(pt0[:], onesr_s[:], ba_s[:],
                                     start=False, stop=True)
            else:
                pt0 = step1(0)
                vt0 = step2p(0)
                cast_slab(1)
            vts = vcopy(0, vt0)
            sc = stats(0, pt0)
            prev = 0

            # ---- steady state: PE order [step2p(g), step4(g-1),
            #      step1(g)] -- step4 only needs vts/mt (never DMA-gated)
            #      and its scaled copies catch sc(g-1) just in time.
            #      vcopy is emitted before stats so it isn't stuck behind
            #      the reciprocal on the DVE FIFO. ----
            for g in range(1, nslab):
                if g == nslab - 1:
                    # last slab: stats first so the tail waits less
                    pt = step1(g)
                    sc_n = stats(g, pt)
                    vt = step2p(g)
                    step4(prev, vts, sc, tail=False)
                    sc = sc_n
                    vts = vcopy(g, vt)
                else:
                    vt = vt_pre.pop(g, None)
                    if vt is None:
                        vt = step2p(g)
                    step4(prev, vts, sc, tail=False)
                    pt = step1(g)
                    cast_slab(g + 1)
                    vts = vcopy(g, vt)
                    sc = stats(g, pt)
                prev = g
            step4(prev, vts, sc, tail=True)

    nc.compile()
    return nc


def _host_prep(inputs, use_fp8=True):
    """Fold weights, swizzle X into fp16 token slabs, shard across cores."""
    h = np.asarray(inputs["hidden_states"], dtype=np.float32)
    b, s, hid = h.shape
    assert hid == HID
    x = np.ascontiguousarray(h.reshape(b * s, hid))
    ntok = b * s
    tok = ntok // N_CORES
    nslab = tok // SLAB

    kv_a_w = np.asarray(inputs["kv_a_w"], np.float32)
    kv_a_b = np.asarray(inputs["kv_a_b"], np.float32)
    kv_norm_w = np.asarray(inputs["kv_norm_w"], np.float32)
    kv_b_w = np.asarray(inputs["kv_b_w"], np.float32)
    kv_b_b = np.asarray(inputs["kv_b_b"], np.float32)
    o_w = np.asarray(inputs["o_w"], np.float32)

    wv = kv_b_w[D:2 * D] * (1.0 + kv_norm_w)[None, :]       # (128, 512)
    wc = wv @ kv_a_w                                        # (128, 2048)
    M = o_w.reshape(HID, 16, D).sum(axis=1)                 # (2048, 128)

    w1t = kv_a_w.T.reshape(NCK, 128, KV).transpose(1, 0, 2)
    if use_fp8:
        w1s = np.ascontiguousarray(w1t * WSCALE).astype(NP_F8)
    else:
        w1s = np.ascontiguousarray(w1t).astype(np.float16)
    wct = np.ascontiguousarray(
        wc.T.reshape(NCK, 128, D).transpose(1, 0, 2)
    ).astype(np.float16)
    mt = np.ascontiguousarray(M.T).astype(np.float16)

    with_ba = bool(np.any(kv_a_b != 0.0))
    with_bv = bool(np.any(kv_b_b[D:2 * D] != 0.0))
    ba_row = np.ascontiguousarray(
        (kv_a_b * (WSCALE if use_fp8 else 1.0)).reshape(1, KV)
    ).astype(np.float16)
    ones_row = np.ones((1, 128), np.float16)
    cv = np.ascontiguousarray((wv @ kv_a_b).reshape(D, 1)).astype(np.float32)
    yb = M @ kv_b_b[D:2 * D]                                # (2048,)
    ybb = np.ascontiguousarray(
        np.broadcast_to(yb[None, :], (128, OUT))).astype(np.float16)

    in_maps = []
    for i in range(N_CORES):
        shard = x[i * tok:(i + 1) * tok]
        xts = np.ascontiguousarray(
            shard.T.reshape(NCK, 128, nslab, SLAB).transpose(2, 1, 0, 3)
        ).astype(np.float16)
        m = {"xts": xts, "w1s": w1s, "wct": wct, "mt": mt}
        if with_ba:
            m["bar"] = ba_row
            m["onesr"] = ones_row
            m["cv"] = cv
        if with_bv:
            m["ybb"] = ybb
        in_maps.append(m)

    def gather(results):
        y = np.concatenate([r["y"] for r in results], axis=0)
        return np.ascontiguousarray(y.reshape(b, s, HID).astype(np.float32))

    return in_maps, gather, with_ba, with_bv, tok


def _run(inputs, trace=False, use_fp8=True, **spmd_kwargs):
    in_maps, gather, with_ba, with_bv, tok = _host_prep(inputs, use_fp8)
    key = (tok, with_ba, with_bv, use_fp8)
    if key not in _NC_CACHE:
        _NC_CACHE[key] = _build_nc(tok, with_ba, with_bv, use_fp8)
    nc = _NC_CACHE[key]
    res = run_bass_kernel_spmd(nc, in_maps, core_ids=list(range(N_CORES)),
                               trace=trace, **spmd_kwargs)
    return gather(res.results), res


def kernel(**inputs) -> np.ndarray:
    y, _ = _run(inputs, trace=False)
    return y
